# revision 18
# baseline (speedup 1.0000x reference)
"""Self-contained Trainium2 Bass kernel for a 2-layer GAT (GATConv x2, PyG-style).

Contract: kernel(**inputs) takes the FULL inputs (x [N,128] f32, edge_index
[2,E] int, W1/att_src1/att_dst1/b1/W2/att_src2/att_dst2/b2) and returns the
FULL [N,128] f32 output, distributing work across 8 NeuronCores internally.

Strategy (graph-parallel, destination-sharded, aggregate-then-project):
  - Because W is shared across nodes, sum_s alpha_s (x_s @ W) = (sum_s
    alpha_s x_s) @ W: each layer aggregates RAW input features per
    destination and applies the dense projection once per 128-dst block.
  - Layer-1 gather tables are host-prepared x rows (128 bf16 = 256 B); the
    per-edge a_src1 logit term is baked into the host-built mask table and
    a_dst1 is a host input, so layer-1 edge gathers start immediately.
  - Destinations are sharded by node-id range across the 8 cores; each core
    owns 6272 padded rank positions = 49 blocks of 128 (dst = SBUF
    partition). Source rows are fetched with dma_gather (int16 indices);
    the two index windows are rank-based: window A = every core's ranks
    0..4095 (blocks 0-31, 32768 rows), window B = ranks 4096.. (blocks
    32-48). Per-core trailing all-pad slots are trimmed via num_idxs_reg.
  - Softmax runs on DVE/ACT per block; aggregation is TensorE diag-matmuls
    D[:,s,:] = diag(alpha[:,s]) accumulated in PSUM giving aggX^T [feat,dst].
  - Per block, layer 1 then computes x2^T = relu(W1^T aggX^T + b1) (one
    matmul), a_src2/a_dst2 (one matmul), PE-transposes x2^T to node-major
    rows, and stages [x2 0:126 bf16 | x2 126:128 fp8e4 | a_src2 bf16] = 256 B.
  - Two AllGathers (after blocks 31 and 48) ship the staged rows; the
    AllGather OUTPUT buffers ARE the layer-2 gather window tables (no
    redundant per-core h recompute or table write).
  - Layer 2 aggregates x2 messages (bf16 cols 0:126 via diag-matmuls; the two
    fp8 cols via a DVE mult+reduce folded into the same PSUM through a
    zero-padded transpose matmul), projects with W2 and writes out^T.
"""

import hashlib
import os as _os
from contextlib import ExitStack

import ml_dtypes
import numpy as np

# ---------------------------------------------------------------------------
# Configuration
# ---------------------------------------------------------------------------

F = 128              # feature dim (all of F_in, H, F_out)
CORES = 8
NEG_SLOPE = 0.2
MASK_NEG = -30000.0
NQUEUES = int(_os.environ.get("GAT_NQUEUES", "4"))
DMA_SCRATCH = int(_os.environ.get("GAT_DMA_SCRATCH", "32768"))
GBUFS = int(_os.environ.get("GAT_GBUFS", "4"))
DBUFS = int(_os.environ.get("GAT_DBUFS", "3"))
GCHUNK = int(_os.environ.get("GAT_GCHUNK", "32"))  # max slots per gather call
SINGLE_PACKET = _os.environ.get("GAT_SINGLE_PACKET", "1") == "1"
SORT_ITERS = int(_os.environ.get("GAT_SORT_ITERS", "3"))


class Cfg:
    def __init__(self, n_nodes, per_core_blocks, blocks_a):
        self.N = n_nodes
        self.NB = per_core_blocks              # blocks of 128 dsts per core
        self.NBA = blocks_a                    # blocks in window/chunk A
        self.NBB = per_core_blocks - blocks_a
        self.PERP = per_core_blocks * 128      # padded positions per core
        self.PER = n_nodes // CORES            # real nodes per core
        assert self.PER * CORES == n_nodes
        assert self.PERP >= self.PER
        self.NPAD = self.PERP * CORES
        self.RA = blocks_a * 128               # ranks in window A per core
        self.ROWSA = self.RA * CORES
        self.ROWSB = (self.PERP - self.RA) * CORES
        assert self.ROWSA - 1 < 32768 and self.ROWSB - 1 < 32768


FULL_CFG = Cfg(50000, 49, 32)


# ---------------------------------------------------------------------------
# Host-side topology preprocessing (pure graph structure, no feature math)
# ---------------------------------------------------------------------------

def _snake_order(w0, w1):
    """Order dsts by (w0, w1) with alternating w1 direction per w0-run, so
    consecutive 128-groups have near-constant (w0, w1)."""
    idx = np.lexsort((w1, w0))
    w0s = w0[idx]
    out = []
    i = 0
    flip = False
    while i < len(idx):
        j = i
        while j < len(idx) and w0s[j] == w0s[i]:
            j += 1
        run = idx[i:j]
        out.append(run[::-1] if flip else run)
        flip = not flip
        i = j
    return np.concatenate(out)


def build_topology(cfg, edge_index):
    """Block structure, gather indices, per-slot source map, masks."""
    N, PER, PERP, NB = cfg.N, cfg.PER, cfg.PERP, cfg.NB
    RA = cfg.RA
    loops = np.arange(N, dtype=np.int64)
    src = np.concatenate([np.asarray(edge_index[0], np.int64), loops])
    dst = np.concatenate([np.asarray(edge_index[1], np.int64), loops])
    core_of = dst // PER          # owner core of each edge (by dst)
    src_core = src // PER

    # per-core local edge lists
    ek = []
    for k in range(CORES):
        m = core_of == k
        ek.append((src[m], dst[m] - PER * k))

    # window A membership of a SOURCE is rank < RA in its own core. Pass 1
    # assigns ranks by total degree (window-independent), freezing window
    # membership; pass 2 re-sorts each window segment by its exact (w0, w1)
    # keys, which cannot change membership, so the keys stay valid.
    rankg = np.empty(N, dtype=np.int64)
    for k in range(CORES):
        s_k, d_k = ek[k]
        w = np.bincount(d_k, minlength=PER)
        order = np.argsort(w, kind="stable")
        rankg[order + PER * k] = np.arange(PER)
    inA = rankg < RA
    for k in range(CORES):
        s_k, d_k = ek[k]
        w0 = np.bincount(d_k[inA[s_k]], minlength=PER)
        w1 = np.bincount(d_k[~inA[s_k]], minlength=PER)
        rk = rankg[PER * k:PER * (k + 1)]
        for lo_r, hi_r in ((0, RA), (RA, PERP)):
            seg = np.nonzero((rk >= lo_r) & (rk < hi_r))[0]  # local dst ids
            if not len(seg):
                continue
            sub = seg[_snake_order(w0[seg], w1[seg])]
            rankg[sub + PER * k] = lo_r + np.arange(len(sub))

    # final per-block maxima from the fixed ranks
    inA = rankg < RA
    S0k = np.zeros((CORES, NB), np.int64)
    S1k = np.zeros((CORES, NB), np.int64)
    w0s, w1s = [], []
    for k in range(CORES):
        s_k, d_k = ek[k]
        w0 = np.bincount(d_k[inA[s_k]], minlength=PER)
        w1 = np.bincount(d_k[~inA[s_k]], minlength=PER)
        w0r = np.zeros(PERP, np.int64)
        w1r = np.zeros(PERP, np.int64)
        w0r[rankg[PER * k:PER * (k + 1)]] = w0
        w1r[rankg[PER * k:PER * (k + 1)]] = w1
        S0k[k] = w0r.reshape(NB, 128).max(1)
        S1k[k] = w1r.reshape(NB, 128).max(1)
        w0s.append(w0)
        w1s.append(w1)

    S0 = np.maximum(S0k.max(0), 1)   # >=1: dummy dsts get one fake A slot
    S1 = S1k.max(0)
    S0k = np.maximum(S0k, 1)
    T = S0 + S1
    MT = int(T.sum())
    IA = int(8 * S0.sum())
    IB = int(8 * S1.sum())

    # position (k*PERP + rank) -> orig id
    pos2orig = np.full(cfg.NPAD, -1, dtype=np.int64)
    ids = np.arange(N, dtype=np.int64)
    pos2orig[(ids // PER) * PERP + rankg] = ids

    # source table rows (rank-window layout)
    rowA = src_core * RA + rankg[src]               # valid when inA[src]
    rowB = src_core * (PERP - RA) + (rankg[src] - RA)

    idxA = np.zeros((CORES, 128, max(IA, 16)), np.int16)
    idxB = np.zeros((CORES, 128, max(IB, 16)), np.int16)
    srcmap = np.full((CORES, 128, MT), -1, np.int64)  # orig src id per slot

    for k in range(CORES):
        m = core_of == k
        e_src, e_dst = src[m], dst[m] - PER * k
        e_rank = rankg[e_src + 0]  # rank of source (global array indexed by id)
        e_inA = inA[e_src]
        e_rowA = rowA[m]
        e_rowB = rowB[m]
        drank = rankg[e_dst + PER * k]
        bucket0 = [[] for _ in range(PERP)]
        bucket1 = [[] for _ in range(PERP)]
        for dr, ia, ra, rb, so in zip(drank, e_inA, e_rowA, e_rowB, e_src):
            if ia:
                bucket0[dr].append((ra, so))
            else:
                bucket1[dr].append((rb, so))
        for p in range(PERP):
            if not bucket0[p] and not bucket1[p]:
                bucket0[p].append((0, pos2orig[0 * PERP + 0]))  # fake: row 0
        aoff = boff = moff = 0
        for b in range(NB):
            s0, s1 = int(S0[b]), int(S1[b])
            flatA = np.zeros(128 * s0, np.int64)
            flatB = np.zeros(128 * s1, np.int64)
            for p in range(128):
                g = 128 * b + p
                for s_i, (r, so) in enumerate(bucket0[g]):
                    flatA[s_i * 128 + p] = r
                    srcmap[k, p, moff + s_i] = so
                for s_i, (r, so) in enumerate(bucket1[g]):
                    flatB[s_i * 128 + p] = r
                    srcmap[k, p, moff + s0 + s_i] = so
            for nfl, arr, tgt, off in ((s0, flatA, idxA, aoff),
                                       (s1, flatB, idxB, boff)):
                if nfl == 0:
                    continue
                cols = 8 * nfl
                wrapped = arr.reshape(cols, 16).T.astype(np.int16)
                tgt[k, :, off:off + cols] = np.tile(wrapped, (8, 1))
            aoff += 8 * s0
            boff += 8 * s1
            moff += s0 + s1

    stats = {
        "real_edges": int(len(src)),
        "padded_edges": int(MT * 128 * CORES),
        "trimmed_edges": int(MT * 128 * CORES),
    }
    return {
        "S0": S0, "S1": S1, "T": T, "IA": IA, "IB": IB, "MT": MT,
        "idxA": idxA, "idxB": idxB, "srcmap": srcmap,
        "rankg": rankg, "pos2orig": pos2orig, "stats": stats,
    }


# ---------------------------------------------------------------------------
# Bass program
# ---------------------------------------------------------------------------

def build_program(cfg, topo):
    import concourse.bacc as bacc
    import concourse.mybir as mybir
    import concourse.tile as tile

    dt = mybir.dt
    S0, S1, T = topo["S0"], topo["S1"], topo["T"]
    IA, IB, MT = topo["IA"], topo["IB"], topo["MT"]
    PERP, NB, NBA = cfg.PERP, cfg.NB, cfg.NBA
    RA, ROWSA, ROWSB = cfg.RA, cfg.ROWSA, cfg.ROWSB

    nc = bacc.Bacc("TRN2", target_bir_lowering=False, debug=False,
                   enable_asserts=False, num_devices=CORES,
                   num_swdge_queues=NQUEUES,
                   dynamic_dma_scratch_size=DMA_SCRATCH)

    # --- kernel I/O ---
    xtabA = nc.dram_tensor("xtabA", [ROWSA, F], dt.bfloat16,
                           kind="ExternalInput")
    xtabB = nc.dram_tensor("xtabB", [ROWSB, F], dt.bfloat16,
                           kind="ExternalInput")
    idxA_d = nc.dram_tensor("idxA", [128, max(IA, 16)], dt.int16,
                            kind="ExternalInput")
    idxB_d = nc.dram_tensor("idxB", [128, max(IB, 16)], dt.int16,
                            kind="ExternalInput")
    mnegA_d = nc.dram_tensor("mnegA", [128, MT], dt.float32,
                             kind="ExternalInput")
    mneg2_d = nc.dram_tensor("mneg2", [128, MT], dt.float32,
                             kind="ExternalInput")
    adst1_d = nc.dram_tensor("adst1", [128, NB], dt.float32,
                             kind="ExternalInput")
    W1_d = nc.dram_tensor("W1b", [F, F], dt.bfloat16, kind="ExternalInput")
    W2_d = nc.dram_tensor("W2b", [F, F], dt.bfloat16, kind="ExternalInput")
    Wa2_d = nc.dram_tensor("Wa2", [F, 2], dt.bfloat16, kind="ExternalInput")
    b1c_d = nc.dram_tensor("b1c", [128, 1], dt.float32, kind="ExternalInput")
    b2c_d = nc.dram_tensor("b2c", [128, 1], dt.float32, kind="ExternalInput")
    eye_d = nc.dram_tensor("eye", [128, 128], dt.bfloat16, kind="ExternalInput")
    out_d = nc.dram_tensor("out", [F, PERP], dt.float32, kind="ExternalOutput")
    DEBUG = _os.environ.get("GAT_DEBUG", "0") == "1"
    if DEBUG:
        dbg_in = nc.dram_tensor("dbg_in", [PERP * F], dt.bfloat16,
                                kind="ExternalOutput")
        dbg_ccA = nc.dram_tensor("dbg_ccA", [ROWSA * F], dt.bfloat16,
                                 kind="ExternalOutput")
        dbg_ad = nc.dram_tensor("dbg_ad", [128, 2 * NB], dt.float32,
                                kind="ExternalOutput")
        dbg_blk = nc.dram_tensor("dbg_blk", [128, 384], dt.float32,
                                 kind="ExternalOutput")
        dbg_blk2 = nc.dram_tensor("dbg_blk2", [128, 260], dt.float32,
                                  kind="ExternalOutput")
        TMAXg = int(T.max())
        dbg_G = nc.dram_tensor("dbg_G", [128, 2 * TMAXg * F], dt.bfloat16,
                               kind="ExternalOutput")

    # --- internal DRAM ---
    cc_inA = nc.dram_tensor("cc_inA", [RA * F], dt.bfloat16)
    cc_inB = nc.dram_tensor("cc_inB", [(PERP - RA) * F], dt.bfloat16)
    ccA = nc.dram_tensor("ccA", [ROWSA * F], dt.bfloat16, addr_space="Shared")
    ccB = nc.dram_tensor("ccB", [ROWSB * F], dt.bfloat16, addr_space="Shared")

    with tile.TileContext(nc) as tc, ExitStack() as ctx:
        P = ctx.enter_context(tc.tile_pool(name="persist", bufs=1))
        gp = ctx.enter_context(tc.tile_pool(name="gp", bufs=GBUFS))
        sp = ctx.enter_context(tc.tile_pool(name="sp", bufs=3))
        wp = ctx.enter_context(tc.tile_pool(name="wp", bufs=DBUFS))
        ab = ctx.enter_context(tc.tile_pool(name="ab", bufs=2))
        xp = ctx.enter_context(tc.tile_pool(name="xp", bufs=2))
        hp = ctx.enter_context(tc.tile_pool(name="hp", bufs=2))
        sdp = ctx.enter_context(tc.tile_pool(name="sdp", bufs=2))
        pq = ctx.enter_context(tc.tile_pool(name="pq", bufs=2, space="PSUM"))
        pr = ctx.enter_context(tc.tile_pool(name="pr", bufs=2, space="PSUM"))
        pt = ctx.enter_context(tc.tile_pool(name="pt", bufs=2, space="PSUM"))

        # persistent SBUF
        idxA_s = P.tile([128, max(IA, 16)], dt.int16)
        idxB_s = P.tile([128, max(IB, 16)], dt.int16)
        mnegA_s = P.tile([128, MT], dt.float32)
        mneg2_s = P.tile([128, MT], dt.float32)
        adst1_s = P.tile([128, NB], dt.float32)
        W1_s = P.tile([F, F], dt.bfloat16)
        W2_s = P.tile([F, F], dt.bfloat16)
        Wa2_s = P.tile([F, 2], dt.bfloat16)
        b1c_s = P.tile([128, 1], dt.float32)
        b2c_s = P.tile([128, 1], dt.float32)
        eye_s = P.tile([128, 128], dt.bfloat16)
        adst2 = P.tile([128, NB], dt.float32)
        asrc2 = P.tile([128, NB], dt.bfloat16)

        # pre-zero gather buffers (trimmed slots must stay finite) and the
        # zero-padded side-matmul lhsT staging tiles
        TMAX = int(T.max())
        for _ in range(GBUFS):
            Gz = gp.tile([128, TMAX, F], dt.bfloat16, tag="G")
            nc.vector.memset(Gz[:], 0)
        for _ in range(2):
            Sz = sdp.tile([128, 128], dt.bfloat16, tag="Spad")
            nc.vector.memset(Sz[:], 0)

        nc.sync.dma_start(idxA_s[:], idxA_d[:])
        nc.sync.dma_start(idxB_s[:], idxB_d[:])
        nc.sync.dma_start(mnegA_s[:], mnegA_d[:])
        nc.sync.dma_start(mneg2_s[:], mneg2_d[:])
        nc.sync.dma_start(adst1_s[:], adst1_d[:])
        nc.sync.dma_start(W1_s[:], W1_d[:])
        nc.sync.dma_start(W2_s[:], W2_d[:])
        nc.sync.dma_start(Wa2_s[:], Wa2_d[:])
        nc.sync.dma_start(b1c_s[:], b1c_d[:])
        nc.sync.dma_start(b2c_s[:], b2c_d[:])
        nc.sync.dma_start(eye_s[:], eye_d[:])

        state = {"q": 0}

        def gather_block(G, b, aoff, boff, tabA, tabB):
            s0, s1 = int(S0[b]), int(S1[b])
            for c0 in range(0, s0, GCHUNK):
                cn = min(GCHUNK, s0 - c0)
                nc.gpsimd.dma_gather(
                    G[:, c0:c0 + cn, :], tabA,
                    idxA_s[:, aoff + 8 * c0:aoff + 8 * (c0 + cn)],
                    128 * cn, 128 * cn, F, queue_num=state["q"] % NQUEUES,
                    single_packet=SINGLE_PACKET)
                state["q"] += 1
            for c0 in range(0, s1, GCHUNK):
                cn = min(GCHUNK, s1 - c0)
                nc.gpsimd.dma_gather(
                    G[:, s0 + c0:s0 + c0 + cn, :], tabB,
                    idxB_s[:, boff + 8 * c0:boff + 8 * (c0 + cn)],
                    128 * cn, 128 * cn, F, queue_num=state["q"] % NQUEUES,
                    single_packet=SINGLE_PACKET)
                state["q"] += 1

        def softmax_D(E, t):
            """logits E [128,t] f32 -> attention diag tensor D [128,t,128]."""
            EL = sp.tile([128, t], dt.float32, tag="EL")
            nc.vector.scalar_tensor_tensor(
                EL[:], E, NEG_SLOPE, E,
                mybir.AluOpType.mult, mybir.AluOpType.max)
            EX = sp.tile([128, t], dt.float32, tag="EX")
            den = sp.tile([128, 1], dt.float32, tag="den")
            nc.scalar.activation(EX[:], EL[:],
                                 mybir.ActivationFunctionType.Exp,
                                 accum_out=den[:])
            rec = sp.tile([128, 1], dt.float32, tag="rec")
            nc.vector.reciprocal(rec[:], den[:])
            alf = sp.tile([128, t], dt.bfloat16, tag="alf")
            nc.scalar.activation(alf[:], EX[:],
                                 mybir.ActivationFunctionType.Copy,
                                 scale=rec[:])
            D = wp.tile([128, t, 128], dt.bfloat16, tag="D")
            nc.vector.tensor_tensor(
                D[:], alf[:].unsqueeze(2).broadcast_to([128, t, 128]),
                eye_s[:].unsqueeze(1).broadcast_to([128, t, 128]),
                mybir.AluOpType.mult)
            return alf, D

        # ------------------------- layer 1 -------------------------
        aoff = boff = moff = 0
        for b in range(NB):
            s0, s1 = int(S0[b]), int(S1[b])
            t = s0 + s1
            G = gp.tile([128, TMAX, F], dt.bfloat16, tag="G")
            gather_block(G, b, aoff, boff, xtabA[:], xtabB[:])
            # logits purely from host inputs: mask-with-baked-a_src + a_dst
            E = sp.tile([128, t], dt.float32, tag="E")
            nc.vector.tensor_scalar_add(E[:], mnegA_s[:, moff:moff + t],
                                        adst1_s[:, b:b + 1])
            alf, D = softmax_D(E[:], t)
            agg = pq.tile([128, 512], dt.float32, tag="agg")
            for s in range(t):
                nc.tensor.matmul(agg[:, 0:128], G[:, s, :], D[:, s, :],
                                 start=(s == 0), stop=(s == t - 1))
            aggXsb = ab.tile([128, 128], dt.bfloat16, tag="aggX")
            nc.scalar.activation(aggXsb[:], agg[:, 0:128],
                                 mybir.ActivationFunctionType.Copy)
            psP = pr.tile([128, 512], dt.float32, tag="psP")
            nc.tensor.matmul(psP[:, 0:128], W1_s[:], aggXsb[:])
            x2T = xp.tile([128, 128], dt.bfloat16, tag="x2T")
            nc.scalar.activation(x2T[:], psP[:, 0:128],
                                 mybir.ActivationFunctionType.Relu,
                                 bias=b1c_s[:])
            # a_src2 / a_dst2 for this block's dsts
            nc.tensor.matmul(psP[:, 128:130], x2T[:], Wa2_s[:])
            nc.vector.tensor_copy(asrc2[:, b:b + 1], psP[:, 128:129])
            nc.vector.tensor_copy(adst2[:, b:b + 1], psP[:, 129:130])
            # x2 rows node-major for the exchange table
            psT = pt.tile([128, 512], dt.float32, tag="psT")
            nc.tensor.matmul(psT[:, 0:128], x2T[:], eye_s[:])
            stage = hp.tile([128, 128], dt.bfloat16, tag="stage")
            nc.scalar.activation(stage[:, 0:126], psT[:, 0:126],
                                 mybir.ActivationFunctionType.Copy)
            st8 = stage[:].bitcast(dt.float8e4)      # [128, 256]
            nc.scalar.activation(st8[:, 252:254], psT[:, 126:128],
                                 mybir.ActivationFunctionType.Copy)
            nc.vector.tensor_copy(stage[:, 127:128], asrc2[:, b:b + 1])
            if b < NBA:
                ccv = cc_inA[128 * b * F:128 * (b + 1) * F]
            else:
                bb = b - NBA
                ccv = cc_inB[128 * bb * F:128 * (bb + 1) * F]
            nc.scalar.dma_start(ccv.rearrange("(p f) -> p f", p=128),
                                stage[:])
            if b == NBA - 1:
                nc.gpsimd.collective_compute(
                    "AllGather", mybir.AluOpType.bypass,
                    replica_groups=[list(range(CORES))],
                    ins=[cc_inA[:].opt()],
                    outs=[ccA[:].rearrange("(k e) -> k e", k=CORES).opt()])
            if b == NB - 1:
                nc.gpsimd.collective_compute(
                    "AllGather", mybir.AluOpType.bypass,
                    replica_groups=[list(range(CORES))],
                    ins=[cc_inB[:].opt()],
                    outs=[ccB[:].rearrange("(k e) -> k e", k=CORES).opt()])
            aoff += 8 * s0
            boff += 8 * s1
            moff += t

        if DEBUG:
            nc.sync.dma_start(dbg_in[0:RA * F], cc_inA[:])
            nc.sync.dma_start(dbg_in[RA * F:], cc_inB[:])
            nc.sync.dma_start(dbg_ccA[:], ccA[:])
            asr32 = sp.tile([128, NB], dt.float32, tag="asr32")
            nc.vector.tensor_copy(asr32[:], asrc2[:])
            nc.sync.dma_start(dbg_ad[:, 0:NB], asr32[:])
            nc.sync.dma_start(dbg_ad[:, NB:], adst2[:])

        # ------------------------- layer 2 -------------------------
        tabA2 = ccA[:].rearrange("(n f) -> n f", f=F)
        tabB2 = ccB[:].rearrange("(n f) -> n f", f=F)
        aoff = boff = moff = 0
        for b in range(NB):
            s0, s1 = int(S0[b]), int(S1[b])
            t = s0 + s1
            G = gp.tile([128, TMAX, F], dt.bfloat16, tag="G")
            gather_block(G, b, aoff, boff, tabA2, tabB2)
            if DEBUG and b in (0, 4):
                j = (0, 4).index(b)
                nc.sync.dma_start(
                    dbg_G[:, j * TMAX * F:j * TMAX * F + t * F],
                    G[:, 0:t, :].rearrange("p t f -> p (t f)"))
            E = sp.tile([128, t], dt.float32, tag="E")
            nc.vector.scalar_tensor_tensor(
                E[:], G[:, 0:t, 127], adst2[:, b:b + 1],
                mneg2_s[:, moff:moff + t],
                mybir.AluOpType.add, mybir.AluOpType.add)
            alf, D = softmax_D(E[:], t)
            if DEBUG and b in (0, 12):
                j = (0, 12).index(b)
                nc.sync.dma_start(dbg_blk[:, 64 * j:64 * j + t], E[:])
                al32 = sp.tile([128, t], dt.float32, tag="al32")
                nc.vector.tensor_copy(al32[:], alf[:])
                nc.sync.dma_start(dbg_blk[:, 128 + 64 * j:128 + 64 * j + t],
                                  al32[:])
                g32 = sp.tile([128, t], dt.float32, tag="g32")
                nc.vector.tensor_copy(g32[:], G[:, 0:t, 127])
                nc.sync.dma_start(dbg_blk[:, 256 + 64 * j:256 + 64 * j + t],
                                  g32[:])
            # the two fp8-packed features: DVE weighted reduce, folded into
            # the same PSUM through a zero-padded transpose matmul. It runs
            # FIRST with start=True so its zero columns clear partitions
            # 0:126 (stale has_written from the recycled bank) and rows
            # 126:128 carry the side sums; the diag matmuls accumulate onto.
            G8 = G[:].bitcast(dt.float8e4)           # [128, TMAX, 256]
            sv = G8[:, 0:t, 252:254].rearrange("p t c -> p c t")
            SM = sp.tile([128, 2, t], dt.float32, tag="SM")
            nc.vector.tensor_tensor(
                SM[:], sv, alf[:].unsqueeze(1).broadcast_to([128, 2, t]),
                mybir.AluOpType.mult)
            Sred = sp.tile([128, 2], dt.float32, tag="Sred")
            nc.vector.tensor_reduce(Sred[:], SM[:], mybir.AxisListType.X,
                                    mybir.AluOpType.add)
            Spad = sdp.tile([128, 128], dt.bfloat16, tag="Spad")
            nc.vector.tensor_copy(Spad[:, 126:128], Sred[:])
            agg = pq.tile([128, 512], dt.float32, tag="agg")
            nc.tensor.matmul(agg[:, 0:128], Spad[:], eye_s[:],
                             start=True, stop=False)
            for s in range(t):
                nc.tensor.matmul(agg[0:126, 0:128], G[:, s, 0:126],
                                 D[:, s, :], start=False, stop=(s == t - 1))
            aggXsb = ab.tile([128, 128], dt.bfloat16, tag="aggX")
            nc.scalar.activation(aggXsb[:], agg[:, 0:128],
                                 mybir.ActivationFunctionType.Copy)
            if DEBUG and b in (0, 12):
                j = (0, 12).index(b)
                nc.sync.dma_start(dbg_blk2[:, 130 * j:130 * j + 2], Sred[:])
                ax32 = sp.tile([128, 128], dt.float32, tag="ax32")
                nc.vector.tensor_copy(ax32[:], aggXsb[:])
                nc.sync.dma_start(dbg_blk2[:, 130 * j + 2:130 * j + 130],
                                  ax32[:])
            psP = pr.tile([128, 512], dt.float32, tag="psP")
            nc.tensor.matmul(psP[:, 0:128], W2_s[:], aggXsb[:])
            oT = xp.tile([128, 128], dt.float32, tag="oT")
            nc.scalar.activation(oT[:], psP[:, 0:128],
                                 mybir.ActivationFunctionType.Relu,
                                 bias=b2c_s[:])
            nc.sync.dma_start(out_d[:, 128 * b:128 * (b + 1)], oT[:])
            aoff += 8 * s0
            boff += 8 * s1
            moff += t

    nc.compile()
    return nc


# ---------------------------------------------------------------------------
# Host orchestration
# ---------------------------------------------------------------------------

def make_inputs(cfg, topo, x, W1, as1, ad1, b1, W2, as2, ad2, b2):
    N, PERP, NB, RA = cfg.N, cfg.PERP, cfg.NB, cfg.RA
    bf16 = ml_dtypes.bfloat16
    pos2orig = topo["pos2orig"]
    rankg = topo["rankg"]
    srcmap = topo["srcmap"]

    def wcols(W, a_s, a_d):
        W = np.asarray(W, np.float64)
        return np.stack([W @ np.asarray(a_s, np.float64),
                         W @ np.asarray(a_d, np.float64)], axis=1)

    xb = np.asarray(x, np.float32).astype(bf16).astype(np.float32)
    a1 = (xb @ wcols(W1, as1, ad1).astype(bf16).astype(np.float32)).astype(
        np.float32)                                   # [N, 2] f32

    # window tables in rank layout
    ids = np.arange(N, dtype=np.int64)
    core = ids // cfg.PER
    xtabA = np.zeros((cfg.ROWSA, F), bf16)
    xtabB = np.zeros((cfg.ROWSB, F), bf16)
    mA = rankg < RA
    xtabA[core[mA] * RA + rankg[mA]] = xb[mA].astype(bf16)
    xtabB[core[~mA] * (PERP - RA) + rankg[~mA] - RA] = xb[~mA].astype(bf16)

    # mnegA: baked a_src1 per slot (or mask); mneg2: plain mask
    mnegA = np.where(srcmap >= 0,
                     a1[np.maximum(srcmap, 0), 0].astype(np.float32),
                     np.float32(MASK_NEG)).astype(np.float32)
    mneg2 = np.where(srcmap >= 0, np.float32(0.0),
                     np.float32(MASK_NEG)).astype(np.float32)

    # adst1 [CORES, 128, NB]
    adst1 = np.zeros((CORES, 128, NB), np.float32)
    for k in range(CORES):
        po = pos2orig[PERP * k:PERP * (k + 1)]
        vals = np.where(po >= 0, a1[np.maximum(po, 0), 1], 0.0)
        adst1[k] = vals.reshape(NB, 128).T

    W1b = np.asarray(W1, np.float32).astype(bf16)
    W2b = np.asarray(W2, np.float32).astype(bf16)
    Wa2 = wcols(W2, as2, ad2).astype(bf16)
    b1c = np.ascontiguousarray(np.asarray(b1, np.float32).reshape(128, 1))
    b2c = np.ascontiguousarray(np.asarray(b2, np.float32).reshape(128, 1))
    eye = np.eye(128, dtype=bf16)

    in_maps = []
    for k in range(CORES):
        in_maps.append({
            "xtabA": xtabA, "xtabB": xtabB,
            "idxA": topo["idxA"][k], "idxB": topo["idxB"][k],
            "mnegA": mnegA[k], "mneg2": mneg2[k],
            "adst1": adst1[k],
            "W1b": W1b, "W2b": W2b, "Wa2": Wa2,
            "b1c": b1c, "b2c": b2c, "eye": eye,
        })
    return in_maps


_CACHE = {}


def _get_program(cfg, edge_index):
    key = hashlib.sha1(np.ascontiguousarray(edge_index).tobytes()).hexdigest()
    if key not in _CACHE:
        topo = build_topology(cfg, edge_index)
        nc = build_program(cfg, topo)
        _CACHE[key] = (topo, nc)
    return _CACHE[key]


def run(cfg, inputs, trace=False):
    from concourse.bass_utils import run_bass_kernel_spmd

    topo, nc = _get_program(cfg, inputs["edge_index"])
    in_maps = make_inputs(
        cfg, topo, inputs["x"],
        inputs["W1"], inputs["att_src1"], inputs["att_dst1"], inputs["b1"],
        inputs["W2"], inputs["att_src2"], inputs["att_dst2"], inputs["b2"])
    res = run_bass_kernel_spmd(nc, in_maps, list(range(CORES)), trace=trace)

    full = np.zeros((cfg.N, F), np.float32)
    pos2orig = topo["pos2orig"]
    for k in range(CORES):
        o = np.asarray(res.results[k]["out"], np.float32).T  # [PERP, F]
        po = pos2orig[cfg.PERP * k:cfg.PERP * (k + 1)]
        m = po >= 0
        full[po[m]] = o[m]
    return full, res


def kernel(**inputs) -> np.ndarray:
    out, _ = run(FULL_CFG, inputs)
    return out


# revision 22
# speedup vs baseline: 1.2565x; 1.2565x over previous
"""Self-contained Trainium2 Bass kernel for a 2-layer GAT (GATConv x2, PyG-style).

Contract: kernel(**inputs) takes the FULL inputs (x [N,128] f32, edge_index
[2,E] int, W1/att_src1/att_dst1/b1/W2/att_src2/att_dst2/b2) and returns the
FULL [N,128] f32 output, distributing work across 8 NeuronCores internally.

Strategy (graph-parallel, destination-sharded, aggregate-then-project):
  - Because W is shared across nodes, sum_s alpha_s (x_s @ W) = (sum_s
    alpha_s x_s) @ W: each layer aggregates RAW input features per
    destination and applies the dense projection once per 128-dst block.
  - Layer-1 gather tables are host-prepared x rows (128 bf16 = 256 B); the
    per-edge a_src1 logit term is baked into the host-built mask table and
    a_dst1 is a host input, so layer-1 edge gathers start immediately.
  - Destinations are sharded by node-id range across the 8 cores; each core
    owns 6272 padded rank positions = 49 blocks of 128 (dst = SBUF
    partition). Source rows are fetched with dma_gather (int16 indices);
    the two index windows are rank-based: window A = every core's ranks
    0..4095 (blocks 0-31, 32768 rows), window B = ranks 4096.. (blocks
    32-48). Per-core trailing all-pad slots are trimmed via num_idxs_reg.
  - Softmax runs on DVE/ACT per block; aggregation is TensorE diag-matmuls
    D[:,s,:] = diag(alpha[:,s]) accumulated in PSUM giving aggX^T [feat,dst].
  - Per block, layer 1 then computes x2^T = relu(W1^T aggX^T + b1) (one
    matmul), a_src2/a_dst2 (one matmul), PE-transposes x2^T to node-major
    rows, and stages [x2 0:126 bf16 | x2 126:128 fp8e4 | a_src2 bf16] = 256 B.
  - Two AllGathers (after blocks 31 and 48) ship the staged rows; the
    AllGather OUTPUT buffers ARE the layer-2 gather window tables (no
    redundant per-core h recompute or table write).
  - Layer 2 aggregates x2 messages (bf16 cols 0:126 via diag-matmuls; the two
    fp8 cols via a DVE mult+reduce folded into the same PSUM through a
    zero-padded transpose matmul), projects with W2 and writes out^T.
"""

import hashlib
import os as _os
from contextlib import ExitStack

import ml_dtypes
import numpy as np

# ---------------------------------------------------------------------------
# Configuration
# ---------------------------------------------------------------------------

F = 128              # feature dim (all of F_in, H, F_out)
CORES = 8
NEG_SLOPE = 0.2
MASK_NEG = -30000.0
NQUEUES = int(_os.environ.get("GAT_NQUEUES", "4"))
DMA_SCRATCH = int(_os.environ.get("GAT_DMA_SCRATCH", "32768"))
GBUFS = int(_os.environ.get("GAT_GBUFS", "4"))
DBUFS = int(_os.environ.get("GAT_DBUFS", "3"))
GCHUNK = int(_os.environ.get("GAT_GCHUNK", "32"))  # max slots per gather call
SINGLE_PACKET = _os.environ.get("GAT_SINGLE_PACKET", "1") == "1"
SORT_ITERS = int(_os.environ.get("GAT_SORT_ITERS", "3"))


class Cfg:
    def __init__(self, n_nodes, per_core_blocks, blocks_a):
        self.N = n_nodes
        self.NB = per_core_blocks              # blocks of 128 dsts per core
        self.NBA = blocks_a                    # blocks in window/chunk A
        self.NBB = per_core_blocks - blocks_a
        self.PERP = per_core_blocks * 128      # padded positions per core
        self.PER = n_nodes // CORES            # real nodes per core
        assert self.PER * CORES == n_nodes
        assert self.PERP >= self.PER
        self.NPAD = self.PERP * CORES
        self.RA = blocks_a * 128               # ranks in window A per core
        self.ROWSA = self.RA * CORES
        self.ROWSB = (self.PERP - self.RA) * CORES
        assert self.ROWSA - 1 < 32768 and self.ROWSB - 1 < 32768


FULL_CFG = Cfg(50000, 49, 32)


# ---------------------------------------------------------------------------
# Host-side topology preprocessing (pure graph structure, no feature math)
# ---------------------------------------------------------------------------

def _snake_order(w0, w1):
    """Order dsts by (w0, w1) with alternating w1 direction per w0-run, so
    consecutive 128-groups have near-constant (w0, w1)."""
    idx = np.lexsort((w1, w0))
    w0s = w0[idx]
    out = []
    i = 0
    flip = False
    while i < len(idx):
        j = i
        while j < len(idx) and w0s[j] == w0s[i]:
            j += 1
        run = idx[i:j]
        out.append(run[::-1] if flip else run)
        flip = not flip
        i = j
    return np.concatenate(out)


def build_topology(cfg, edge_index):
    """Block structure, gather indices, per-slot source map, masks.

    Destinations are dealt round-robin across cores by sorted degree so all
    cores share one (w0, w1) block profile; self-loops are excluded (the
    self slot is a static per-block DMA, slot 0)."""
    N, PER, PERP, NB = cfg.N, cfg.PER, cfg.PERP, cfg.NB
    RA = cfg.RA
    src = np.asarray(edge_index[0], np.int64)
    dst = np.asarray(edge_index[1], np.int64)

    # pass 1: global total-degree sort -> deal (core, rank); freeze windows
    w = np.bincount(dst, minlength=N)
    order = np.argsort(w, kind="stable")            # ascending degree
    core_of_node = np.empty(N, np.int64)
    rankg = np.empty(N, np.int64)
    core_of_node[order] = np.arange(N) % CORES
    rankg[order] = np.arange(N) // CORES
    inA = rankg < RA

    # pass 2: exact (w0, w1) per dst under frozen windows; redeal within
    # each window segment globally (keeps membership, so keys stay exact)
    w0 = np.bincount(dst[inA[src]], minlength=N)
    w1 = np.bincount(dst[~inA[src]], minlength=N)
    for seg_mask, lo_r in ((inA, 0), (~inA, RA)):
        ids = np.nonzero(seg_mask)[0]
        sub = ids[_snake_order(w0[ids], w1[ids])]
        core_of_node[sub] = np.arange(len(sub)) % CORES
        rankg[sub] = lo_r + np.arange(len(sub)) // CORES
    assert np.array_equal(inA, rankg < RA)

    # per-block maxima (near-identical across cores by construction)
    S0k = np.zeros((CORES, NB), np.int64)
    S1k = np.zeros((CORES, NB), np.int64)
    for k in range(CORES):
        sel = core_of_node == k
        w0r = np.zeros(PERP, np.int64)
        w1r = np.zeros(PERP, np.int64)
        w0r[rankg[sel]] = w0[sel]
        w1r[rankg[sel]] = w1[sel]
        S0k[k] = w0r.reshape(NB, 128).max(1)
        S1k[k] = w1r.reshape(NB, 128).max(1)
    S0 = S0k.max(0)
    S1 = S1k.max(0)
    T = 1 + S0 + S1                                  # slot 0 = static self
    MT = int(T.sum())
    IA = int(8 * S0.sum())
    IB = int(8 * S1.sum())

    pos2orig = np.full(cfg.NPAD, -1, dtype=np.int64)
    ids = np.arange(N, dtype=np.int64)
    pos2orig[core_of_node * PERP + rankg] = ids

    rowA = core_of_node[src] * RA + rankg[src]
    rowB = core_of_node[src] * (PERP - RA) + (rankg[src] - RA)

    idxA = np.zeros((CORES, 128, max(IA, 16)), np.int16)
    idxB = np.zeros((CORES, 128, max(IB, 16)), np.int16)
    srcmap = np.full((CORES, 128, MT), -1, np.int64)  # orig src id per slot
    e_core = core_of_node[dst]

    for k in range(CORES):
        m = e_core == k
        e_src = src[m]
        e_inA = inA[e_src]
        e_rowA = rowA[m]
        e_rowB = rowB[m]
        drank = rankg[dst[m]]
        bucket0 = [[] for _ in range(PERP)]
        bucket1 = [[] for _ in range(PERP)]
        for dr, ia, ra, rb, so in zip(drank, e_inA, e_rowA, e_rowB, e_src):
            if ia:
                bucket0[dr].append((ra, so))
            else:
                bucket1[dr].append((rb, so))
        aoff = boff = moff = 0
        for b in range(NB):
            s0, s1 = int(S0[b]), int(S1[b])
            flatA = np.zeros(128 * s0, np.int64)
            flatB = np.zeros(128 * s1, np.int64)
            for p in range(128):
                g = 128 * b + p
                po = pos2orig[k * PERP + g]
                srcmap[k, p, moff] = max(po, 0)  # self slot (dummy->0)
                for s_i, (r, so) in enumerate(bucket0[g]):
                    flatA[s_i * 128 + p] = r
                    srcmap[k, p, moff + 1 + s_i] = so
                for s_i, (r, so) in enumerate(bucket1[g]):
                    flatB[s_i * 128 + p] = r
                    srcmap[k, p, moff + 1 + s0 + s_i] = so
            for nfl, arr, tgt, off in ((s0, flatA, idxA, aoff),
                                       (s1, flatB, idxB, boff)):
                if nfl == 0:
                    continue
                cols = 8 * nfl
                wrapped = arr.reshape(cols, 16).T.astype(np.int16)
                tgt[k, :, off:off + cols] = np.tile(wrapped, (8, 1))
            aoff += 8 * s0
            boff += 8 * s1
            moff += 1 + s0 + s1
    stats = {
        "real_edges": int(len(src) + N),
        "padded_edges": int(MT * 128 * CORES),
        "trimmed_edges": int(MT * 128 * CORES),
    }
    return {
        "S0": S0, "S1": S1, "T": T, "IA": IA, "IB": IB, "MT": MT,
        "idxA": idxA, "idxB": idxB, "srcmap": srcmap,
        "rankg": rankg, "core_of_node": core_of_node,
        "pos2orig": pos2orig, "stats": stats,
    }


# ---------------------------------------------------------------------------
# Bass program
# ---------------------------------------------------------------------------

def build_program(cfg, topo):
    import concourse.bacc as bacc
    import concourse.mybir as mybir
    import concourse.tile as tile

    dt = mybir.dt
    S0, S1, T = topo["S0"], topo["S1"], topo["T"]
    IA, IB, MT = topo["IA"], topo["IB"], topo["MT"]
    PERP, NB, NBA = cfg.PERP, cfg.NB, cfg.NBA
    RA, ROWSA, ROWSB = cfg.RA, cfg.ROWSA, cfg.ROWSB

    nc = bacc.Bacc("TRN2", target_bir_lowering=False, debug=False,
                   enable_asserts=False, num_devices=CORES,
                   num_swdge_queues=NQUEUES,
                   dynamic_dma_scratch_size=DMA_SCRATCH)

    # --- kernel I/O ---
    xtabA = nc.dram_tensor("xtabA", [ROWSA, F], dt.bfloat16,
                           kind="ExternalInput")
    xtabB = nc.dram_tensor("xtabB", [ROWSB, F], dt.bfloat16,
                           kind="ExternalInput")
    xtabOwn = nc.dram_tensor("xtabOwn", [PERP, F], dt.bfloat16,
                             kind="ExternalInput")
    idxA_d = nc.dram_tensor("idxA", [128, max(IA, 16)], dt.int16,
                            kind="ExternalInput")
    idxB_d = nc.dram_tensor("idxB", [128, max(IB, 16)], dt.int16,
                            kind="ExternalInput")
    mnegA_d = nc.dram_tensor("mnegA", [128, MT], dt.float32,
                             kind="ExternalInput")
    mneg2_d = nc.dram_tensor("mneg2", [128, MT], dt.float32,
                             kind="ExternalInput")
    adst1_d = nc.dram_tensor("adst1", [128, NB], dt.float32,
                             kind="ExternalInput")
    W1_d = nc.dram_tensor("W1b", [F, F], dt.bfloat16, kind="ExternalInput")
    W2_d = nc.dram_tensor("W2b", [F, F], dt.bfloat16, kind="ExternalInput")
    Wa2_d = nc.dram_tensor("Wa2", [F, 2], dt.bfloat16, kind="ExternalInput")
    b1c_d = nc.dram_tensor("b1c", [128, 1], dt.float32, kind="ExternalInput")
    b2c_d = nc.dram_tensor("b2c", [128, 1], dt.float32, kind="ExternalInput")
    eye_d = nc.dram_tensor("eye", [128, 128], dt.bfloat16, kind="ExternalInput")
    out_d = nc.dram_tensor("out", [F, PERP], dt.float32, kind="ExternalOutput")
    DEBUG = _os.environ.get("GAT_DEBUG", "0") == "1"
    if DEBUG:
        dbg_in = nc.dram_tensor("dbg_in", [PERP * F], dt.bfloat16,
                                kind="ExternalOutput")
        dbg_ccA = nc.dram_tensor("dbg_ccA", [ROWSA * F], dt.bfloat16,
                                 kind="ExternalOutput")
        dbg_ad = nc.dram_tensor("dbg_ad", [128, 2 * NB], dt.float32,
                                kind="ExternalOutput")
        dbg_blk = nc.dram_tensor("dbg_blk", [128, 384], dt.float32,
                                 kind="ExternalOutput")
        dbg_blk2 = nc.dram_tensor("dbg_blk2", [128, 260], dt.float32,
                                  kind="ExternalOutput")
        TMAXg = int(T.max())
        dbg_G = nc.dram_tensor("dbg_G", [128, 2 * TMAXg * F], dt.bfloat16,
                               kind="ExternalOutput")

    # --- internal DRAM ---
    cc_inA = nc.dram_tensor("cc_inA", [RA * F], dt.bfloat16)
    cc_inB = nc.dram_tensor("cc_inB", [(PERP - RA) * F], dt.bfloat16)
    ccA = nc.dram_tensor("ccA", [ROWSA * F], dt.bfloat16, addr_space="Shared")
    ccB = nc.dram_tensor("ccB", [ROWSB * F], dt.bfloat16, addr_space="Shared")

    with tile.TileContext(nc) as tc, ExitStack() as ctx:
        P = ctx.enter_context(tc.tile_pool(name="persist", bufs=1))
        gp = ctx.enter_context(tc.tile_pool(name="gp", bufs=GBUFS))
        sp = ctx.enter_context(tc.tile_pool(name="sp", bufs=3))
        wp = ctx.enter_context(tc.tile_pool(name="wp", bufs=DBUFS))
        ab = ctx.enter_context(tc.tile_pool(name="ab", bufs=2))
        xp = ctx.enter_context(tc.tile_pool(name="xp", bufs=2))
        hp = ctx.enter_context(tc.tile_pool(name="hp", bufs=2))
        sdp = ctx.enter_context(tc.tile_pool(name="sdp", bufs=2))
        pq = ctx.enter_context(tc.tile_pool(name="pq", bufs=2, space="PSUM"))
        pr = ctx.enter_context(tc.tile_pool(name="pr", bufs=2, space="PSUM"))
        pt = ctx.enter_context(tc.tile_pool(name="pt", bufs=2, space="PSUM"))

        # persistent SBUF
        idxA_s = P.tile([128, max(IA, 16)], dt.int16)
        idxB_s = P.tile([128, max(IB, 16)], dt.int16)
        mnegA_s = P.tile([128, MT], dt.float32)
        mneg2_s = P.tile([128, MT], dt.float32)
        adst1_s = P.tile([128, NB], dt.float32)
        W1_s = P.tile([F, F], dt.bfloat16)
        W2_s = P.tile([F, F], dt.bfloat16)
        Wa2_s = P.tile([F, 2], dt.bfloat16)
        b1c_s = P.tile([128, 1], dt.float32)
        b2c_s = P.tile([128, 1], dt.float32)
        eye_s = P.tile([128, 128], dt.bfloat16)
        adst2 = P.tile([128, NB], dt.float32)
        asrc2 = P.tile([128, NB], dt.bfloat16)

        # pre-zero gather buffers (trimmed slots must stay finite) and the
        # zero-padded side-matmul lhsT staging tiles
        TMAX = int(T.max())
        for _ in range(GBUFS):
            Gz = gp.tile([128, TMAX, F], dt.bfloat16, tag="G")
            nc.vector.memset(Gz[:], 0)
        for _ in range(2):
            Sz = sdp.tile([128, 128], dt.bfloat16, tag="Spad")
            nc.vector.memset(Sz[:], 0)

        nc.sync.dma_start(idxA_s[:], idxA_d[:])
        nc.sync.dma_start(idxB_s[:], idxB_d[:])
        nc.sync.dma_start(mnegA_s[:], mnegA_d[:])
        nc.sync.dma_start(mneg2_s[:], mneg2_d[:])
        nc.sync.dma_start(adst1_s[:], adst1_d[:])
        nc.sync.dma_start(W1_s[:], W1_d[:])
        nc.sync.dma_start(W2_s[:], W2_d[:])
        nc.sync.dma_start(Wa2_s[:], Wa2_d[:])
        nc.sync.dma_start(b1c_s[:], b1c_d[:])
        nc.sync.dma_start(b2c_s[:], b2c_d[:])
        nc.sync.dma_start(eye_s[:], eye_d[:])

        state = {"q": 0}

        def gather_block(G, b, aoff, boff, tabA, tabB, own, own_b):
            s0, s1 = int(S0[b]), int(S1[b])
            nc.sync.dma_start(G[:, 0, :],
                              own[128 * own_b * F:128 * (own_b + 1) * F]
                              .rearrange("(p f) -> p f", p=128))
            for c0 in range(0, s0, GCHUNK):
                cn = min(GCHUNK, s0 - c0)
                nc.gpsimd.dma_gather(
                    G[:, 1 + c0:1 + c0 + cn, :], tabA,
                    idxA_s[:, aoff + 8 * c0:aoff + 8 * (c0 + cn)],
                    128 * cn, 128 * cn, F, queue_num=state["q"] % NQUEUES,
                    single_packet=SINGLE_PACKET)
                state["q"] += 1
            for c0 in range(0, s1, GCHUNK):
                cn = min(GCHUNK, s1 - c0)
                nc.gpsimd.dma_gather(
                    G[:, 1 + s0 + c0:1 + s0 + c0 + cn, :], tabB,
                    idxB_s[:, boff + 8 * c0:boff + 8 * (c0 + cn)],
                    128 * cn, 128 * cn, F, queue_num=state["q"] % NQUEUES,
                    single_packet=SINGLE_PACKET)
                state["q"] += 1

        def softmax_D(E, t):
            """logits E [128,t] f32 -> attention diag tensor D [128,t,128]."""
            EL = sp.tile([128, t], dt.float32, tag="EL")
            nc.vector.scalar_tensor_tensor(
                EL[:], E, NEG_SLOPE, E,
                mybir.AluOpType.mult, mybir.AluOpType.max)
            EX = sp.tile([128, t], dt.float32, tag="EX")
            den = sp.tile([128, 1], dt.float32, tag="den")
            nc.scalar.activation(EX[:], EL[:],
                                 mybir.ActivationFunctionType.Exp,
                                 accum_out=den[:])
            rec = sp.tile([128, 1], dt.float32, tag="rec")
            nc.vector.reciprocal(rec[:], den[:])
            alf = sp.tile([128, t], dt.bfloat16, tag="alf")
            nc.scalar.activation(alf[:], EX[:],
                                 mybir.ActivationFunctionType.Copy,
                                 scale=rec[:])
            D = wp.tile([128, t, 128], dt.bfloat16, tag="D")
            nc.vector.tensor_tensor(
                D[:], alf[:].unsqueeze(2).broadcast_to([128, t, 128]),
                eye_s[:].unsqueeze(1).broadcast_to([128, t, 128]),
                mybir.AluOpType.mult)
            return alf, D

        # ------------------------- layer 1 -------------------------
        xtabOwn_f = xtabOwn[:].rearrange("n f -> (n f)")
        aoff = boff = moff = 0
        for b in range(NB):
            s0, s1 = int(S0[b]), int(S1[b])
            t = 1 + s0 + s1
            G = gp.tile([128, TMAX, F], dt.bfloat16, tag="G")
            gather_block(G, b, aoff, boff, xtabA[:], xtabB[:], xtabOwn_f, b)
            # logits purely from host inputs: mask-with-baked-a_src + a_dst
            E = sp.tile([128, t], dt.float32, tag="E")
            nc.vector.tensor_scalar_add(E[:], mnegA_s[:, moff:moff + t],
                                        adst1_s[:, b:b + 1])
            alf, D = softmax_D(E[:], t)
            agg = pq.tile([128, 512], dt.float32, tag="agg")
            for s in range(t):
                nc.tensor.matmul(agg[:, 0:128], G[:, s, :], D[:, s, :],
                                 start=(s == 0), stop=(s == t - 1))
            aggXsb = ab.tile([128, 128], dt.bfloat16, tag="aggX")
            nc.scalar.activation(aggXsb[:], agg[:, 0:128],
                                 mybir.ActivationFunctionType.Copy)
            psP = pr.tile([128, 512], dt.float32, tag="psP")
            nc.tensor.matmul(psP[:, 0:128], W1_s[:], aggXsb[:])
            x2T = xp.tile([128, 128], dt.bfloat16, tag="x2T")
            nc.scalar.activation(x2T[:], psP[:, 0:128],
                                 mybir.ActivationFunctionType.Relu,
                                 bias=b1c_s[:])
            # a_src2 / a_dst2 for this block's dsts
            nc.tensor.matmul(psP[:, 128:130], x2T[:], Wa2_s[:])
            nc.vector.tensor_copy(asrc2[:, b:b + 1], psP[:, 128:129])
            nc.vector.tensor_copy(adst2[:, b:b + 1], psP[:, 129:130])
            # x2 rows node-major for the exchange table
            psT = pt.tile([128, 512], dt.float32, tag="psT")
            nc.tensor.matmul(psT[:, 0:128], x2T[:], eye_s[:])
            stage = hp.tile([128, 128], dt.bfloat16, tag="stage")
            nc.scalar.activation(stage[:, 0:126], psT[:, 0:126],
                                 mybir.ActivationFunctionType.Copy)
            st8 = stage[:].bitcast(dt.float8e4)      # [128, 256]
            nc.scalar.activation(st8[:, 252:254], psT[:, 126:128],
                                 mybir.ActivationFunctionType.Copy)
            nc.vector.tensor_copy(stage[:, 127:128], asrc2[:, b:b + 1])
            if b < NBA:
                ccv = cc_inA[128 * b * F:128 * (b + 1) * F]
            else:
                bb = b - NBA
                ccv = cc_inB[128 * bb * F:128 * (bb + 1) * F]
            nc.scalar.dma_start(ccv.rearrange("(p f) -> p f", p=128),
                                stage[:])
            if b == NBA - 1:
                nc.gpsimd.collective_compute(
                    "AllGather", mybir.AluOpType.bypass,
                    replica_groups=[list(range(CORES))],
                    ins=[cc_inA[:].opt()],
                    outs=[ccA[:].rearrange("(k e) -> k e", k=CORES).opt()])
            if b == NB - 1:
                nc.gpsimd.collective_compute(
                    "AllGather", mybir.AluOpType.bypass,
                    replica_groups=[list(range(CORES))],
                    ins=[cc_inB[:].opt()],
                    outs=[ccB[:].rearrange("(k e) -> k e", k=CORES).opt()])
            aoff += 8 * s0
            boff += 8 * s1
            moff += t

        if DEBUG:
            nc.sync.dma_start(dbg_in[0:RA * F], cc_inA[:])
            nc.sync.dma_start(dbg_in[RA * F:], cc_inB[:])
            nc.sync.dma_start(dbg_ccA[:], ccA[:])
            asr32 = sp.tile([128, NB], dt.float32, tag="asr32")
            nc.vector.tensor_copy(asr32[:], asrc2[:])
            nc.sync.dma_start(dbg_ad[:, 0:NB], asr32[:])
            nc.sync.dma_start(dbg_ad[:, NB:], adst2[:])

        # ------------------------- layer 2 -------------------------
        tabA2 = ccA[:].rearrange("(n f) -> n f", f=F)
        tabB2 = ccB[:].rearrange("(n f) -> n f", f=F)
        aoff = boff = moff = 0
        for b in range(NB):
            s0, s1 = int(S0[b]), int(S1[b])
            t = 1 + s0 + s1
            G = gp.tile([128, TMAX, F], dt.bfloat16, tag="G")
            if b < NBA:
                own2, own_b = cc_inA, b
            else:
                own2, own_b = cc_inB, b - NBA
            gather_block(G, b, aoff, boff, tabA2, tabB2, own2[:], own_b)
            if DEBUG and b in (0, 4):
                j = (0, 4).index(b)
                nc.sync.dma_start(
                    dbg_G[:, j * TMAX * F:j * TMAX * F + t * F],
                    G[:, 0:t, :].rearrange("p t f -> p (t f)"))
            E = sp.tile([128, t], dt.float32, tag="E")
            nc.vector.scalar_tensor_tensor(
                E[:], G[:, 0:t, 127], adst2[:, b:b + 1],
                mneg2_s[:, moff:moff + t],
                mybir.AluOpType.add, mybir.AluOpType.add)
            alf, D = softmax_D(E[:], t)
            if DEBUG and b in (0, 12):
                j = (0, 12).index(b)
                nc.sync.dma_start(dbg_blk[:, 64 * j:64 * j + t], E[:])
                al32 = sp.tile([128, t], dt.float32, tag="al32")
                nc.vector.tensor_copy(al32[:], alf[:])
                nc.sync.dma_start(dbg_blk[:, 128 + 64 * j:128 + 64 * j + t],
                                  al32[:])
                g32 = sp.tile([128, t], dt.float32, tag="g32")
                nc.vector.tensor_copy(g32[:], G[:, 0:t, 127])
                nc.sync.dma_start(dbg_blk[:, 256 + 64 * j:256 + 64 * j + t],
                                  g32[:])
            # the two fp8-packed features: DVE weighted reduce, folded into
            # the same PSUM through a zero-padded transpose matmul. It runs
            # FIRST with start=True so its zero columns clear partitions
            # 0:126 (stale has_written from the recycled bank) and rows
            # 126:128 carry the side sums; the diag matmuls accumulate onto.
            G8 = G[:].bitcast(dt.float8e4)           # [128, TMAX, 256]
            sv = G8[:, 0:t, 252:254].rearrange("p t c -> p c t")
            SM = sp.tile([128, 2, t], dt.float32, tag="SM")
            nc.vector.tensor_tensor(
                SM[:], sv, alf[:].unsqueeze(1).broadcast_to([128, 2, t]),
                mybir.AluOpType.mult)
            Sred = sp.tile([128, 2], dt.float32, tag="Sred")
            nc.vector.tensor_reduce(Sred[:], SM[:], mybir.AxisListType.X,
                                    mybir.AluOpType.add)
            Spad = sdp.tile([128, 128], dt.bfloat16, tag="Spad")
            nc.vector.tensor_copy(Spad[:, 126:128], Sred[:])
            agg = pq.tile([128, 512], dt.float32, tag="agg")
            nc.tensor.matmul(agg[:, 0:128], Spad[:], eye_s[:],
                             start=True, stop=False)
            for s in range(t):
                nc.tensor.matmul(agg[0:126, 0:128], G[:, s, 0:126],
                                 D[:, s, :], start=False, stop=(s == t - 1))
            aggXsb = ab.tile([128, 128], dt.bfloat16, tag="aggX")
            nc.scalar.activation(aggXsb[:], agg[:, 0:128],
                                 mybir.ActivationFunctionType.Copy)
            if DEBUG and b in (0, 12):
                j = (0, 12).index(b)
                nc.sync.dma_start(dbg_blk2[:, 130 * j:130 * j + 2], Sred[:])
                ax32 = sp.tile([128, 128], dt.float32, tag="ax32")
                nc.vector.tensor_copy(ax32[:], aggXsb[:])
                nc.sync.dma_start(dbg_blk2[:, 130 * j + 2:130 * j + 130],
                                  ax32[:])
            psP = pr.tile([128, 512], dt.float32, tag="psP")
            nc.tensor.matmul(psP[:, 0:128], W2_s[:], aggXsb[:])
            oT = xp.tile([128, 128], dt.float32, tag="oT")
            nc.scalar.activation(oT[:], psP[:, 0:128],
                                 mybir.ActivationFunctionType.Relu,
                                 bias=b2c_s[:])
            nc.sync.dma_start(out_d[:, 128 * b:128 * (b + 1)], oT[:])
            aoff += 8 * s0
            boff += 8 * s1
            moff += t

    nc.compile()
    return nc


# ---------------------------------------------------------------------------
# Host orchestration
# ---------------------------------------------------------------------------

def make_inputs(cfg, topo, x, W1, as1, ad1, b1, W2, as2, ad2, b2):
    N, PERP, NB, RA = cfg.N, cfg.PERP, cfg.NB, cfg.RA
    bf16 = ml_dtypes.bfloat16
    pos2orig = topo["pos2orig"]
    rankg = topo["rankg"]
    srcmap = topo["srcmap"]

    def wcols(W, a_s, a_d):
        W = np.asarray(W, np.float64)
        return np.stack([W @ np.asarray(a_s, np.float64),
                         W @ np.asarray(a_d, np.float64)], axis=1)

    xb = np.asarray(x, np.float32).astype(bf16).astype(np.float32)
    a1 = (xb @ wcols(W1, as1, ad1).astype(bf16).astype(np.float32)).astype(
        np.float32)                                   # [N, 2] f32

    # window tables in rank layout
    core = topo["core_of_node"]
    xtabA = np.zeros((cfg.ROWSA, F), bf16)
    xtabB = np.zeros((cfg.ROWSB, F), bf16)
    mA = rankg < RA
    xtabA[core[mA] * RA + rankg[mA]] = xb[mA].astype(bf16)
    xtabB[core[~mA] * (PERP - RA) + rankg[~mA] - RA] = xb[~mA].astype(bf16)
    xtabOwn = np.zeros((CORES, PERP, F), bf16)
    for k in range(CORES):
        po = pos2orig[PERP * k:PERP * (k + 1)]
        mv = po >= 0
        xtabOwn[k][mv] = xb[po[mv]].astype(bf16)

    # mnegA: baked a_src1 per slot (or mask); mneg2: plain mask
    mnegA = np.where(srcmap >= 0,
                     a1[np.maximum(srcmap, 0), 0].astype(np.float32),
                     np.float32(MASK_NEG)).astype(np.float32)
    mneg2 = np.where(srcmap >= 0, np.float32(0.0),
                     np.float32(MASK_NEG)).astype(np.float32)

    # adst1 [CORES, 128, NB]
    adst1 = np.zeros((CORES, 128, NB), np.float32)
    for k in range(CORES):
        po = pos2orig[PERP * k:PERP * (k + 1)]
        vals = np.where(po >= 0, a1[np.maximum(po, 0), 1], 0.0)
        adst1[k] = vals.reshape(NB, 128).T

    W1b = np.asarray(W1, np.float32).astype(bf16)
    W2b = np.asarray(W2, np.float32).astype(bf16)
    Wa2 = wcols(W2, as2, ad2).astype(bf16)
    b1c = np.ascontiguousarray(np.asarray(b1, np.float32).reshape(128, 1))
    b2c = np.ascontiguousarray(np.asarray(b2, np.float32).reshape(128, 1))
    eye = np.eye(128, dtype=bf16)

    in_maps = []
    for k in range(CORES):
        in_maps.append({
            "xtabA": xtabA, "xtabB": xtabB, "xtabOwn": xtabOwn[k],
            "idxA": topo["idxA"][k], "idxB": topo["idxB"][k],
            "mnegA": mnegA[k], "mneg2": mneg2[k],
            "adst1": adst1[k],
            "W1b": W1b, "W2b": W2b, "Wa2": Wa2,
            "b1c": b1c, "b2c": b2c, "eye": eye,
        })
    return in_maps


_CACHE = {}


def _get_program(cfg, edge_index):
    key = hashlib.sha1(np.ascontiguousarray(edge_index).tobytes()).hexdigest()
    if key not in _CACHE:
        topo = build_topology(cfg, edge_index)
        nc = build_program(cfg, topo)
        _CACHE[key] = (topo, nc)
    return _CACHE[key]


def run(cfg, inputs, trace=False):
    from concourse.bass_utils import run_bass_kernel_spmd

    topo, nc = _get_program(cfg, inputs["edge_index"])
    in_maps = make_inputs(
        cfg, topo, inputs["x"],
        inputs["W1"], inputs["att_src1"], inputs["att_dst1"], inputs["b1"],
        inputs["W2"], inputs["att_src2"], inputs["att_dst2"], inputs["b2"])
    res = run_bass_kernel_spmd(nc, in_maps, list(range(CORES)), trace=trace)

    full = np.zeros((cfg.N, F), np.float32)
    pos2orig = topo["pos2orig"]
    for k in range(CORES):
        o = np.asarray(res.results[k]["out"], np.float32).T  # [PERP, F]
        po = pos2orig[cfg.PERP * k:cfg.PERP * (k + 1)]
        m = po >= 0
        full[po[m]] = o[m]
    return full, res


def kernel(**inputs) -> np.ndarray:
    out, _ = run(FULL_CFG, inputs)
    return out
